# revision 15
# baseline (speedup 1.0000x reference)
"""Trainium2 Bass kernel for nn_Pooling_23974507446587 (gnn_message_passing).

Data-parallel over nodes on 8 NeuronCores. Each core processes the shared
first-1024 rows (needed for scatter_mean -> semb, replicated so no collective
is needed) plus its own 6144-row slice: 7168 rows/core. The MLP (99% of
FLOPs), the N x C distance matrix, the top-k selection, and semb are computed
on device in fp32; the host does index plumbing and the O(E) log/exp edge
normalization on device-produced squared distances.
"""
import numpy as np

import concourse.bacc as bacc
import concourse.bass as bass
import concourse.mybir as mybir
from concourse.tile import TileContext
from concourse.bass_utils import run_bass_kernel_spmd
from concourse.masks import make_identity

F32 = mybir.dt.float32
F16 = mybir.dt.float16
U32 = mybir.dt.uint32
AF = mybir.ActivationFunctionType
ALU = mybir.AluOpType

N = 50000
D = 256
H = 1024
EMB = 24
C = 1024
EPS = 1e-12
NCORES = 8
SH = 1024            # shared rows (cluster[0] < 1024)
OWN = 6144           # own rows per core
R = SH + OWN         # 7168 rows per core
MT = 448             # MLP node-tile width (PSUM bank = 512 fp32)
NMT = R // MT        # 16
TK = R // 128        # 56 topk tiles


GELU_C = float(np.sqrt(2.0 / np.pi))
GELU_K = GELU_C * 0.044715


def build_nc(rows=R, gelu_decomp=False):
    nmt = rows // MT
    tk = rows // 128
    nc = bacc.Bacc("TRN2", target_bir_lowering=False, debug=False,
                   enable_asserts=False, num_devices=NCORES)

    xT = nc.dram_tensor("xT", [D, rows], F32, kind="ExternalInput").ap()
    w1 = nc.dram_tensor("w1", [D, H], F32, kind="ExternalInput").ap()
    w2hi = nc.dram_tensor("w2hi", [H, H], F16, kind="ExternalInput").ap()
    w2lo = nc.dram_tensor("w2lo", [H, H], F16, kind="ExternalInput").ap()
    w3hi = nc.dram_tensor("w3hi", [H, H], F16, kind="ExternalInput").ap()
    w3lo = nc.dram_tensor("w3lo", [H, H], F16, kind="ExternalInput").ap()
    w4hi = nc.dram_tensor("w4hi", [H, EMB], F16, kind="ExternalInput").ap()
    w4lo = nc.dram_tensor("w4lo", [H, EMB], F16, kind="ExternalInput").ap()
    b1 = nc.dram_tensor("b1", [128, H // 128], F32, kind="ExternalInput").ap()
    b2 = nc.dram_tensor("b2", [128, H // 128], F32, kind="ExternalInput").ap()
    b3 = nc.dram_tensor("b3", [128, H // 128], F32, kind="ExternalInput").ap()
    b4 = nc.dram_tensor("b4", [EMB, 1], F32, kind="ExternalInput").ap()
    kt = nc.dram_tensor("kt", [C, C], F32, kind="ExternalInput").ap()
    cntr = nc.dram_tensor("cntr", [EMB, C], F32, kind="ExternalInput").ap()
    onesr = nc.dram_tensor("onesr", [1, rows], F32, kind="ExternalInput").ap()

    o_emb = nc.dram_tensor("o_emb", [rows, EMB], F32, kind="ExternalOutput").ap()
    o_semb = nc.dram_tensor("o_semb", [C, EMB], F32, kind="ExternalOutput").ap()
    o_midx = nc.dram_tensor("o_midx", [rows, 8], U32, kind="ExternalOutput").ap()
    o_d2 = nc.dram_tensor("o_d2", [rows, 8], F32, kind="ExternalOutput").ap()
    o_d2c = nc.dram_tensor("o_d2c", [C, C], F32, kind="ExternalOutput").ap()
    o_sidx = nc.dram_tensor("o_sidx", [C, 16], U32, kind="ExternalOutput").ap()
    o_sd2 = nc.dram_tensor("o_sd2", [C, 16], F32, kind="ExternalOutput").ap()

    KC = H // 128   # 8 feature chunks

    with TileContext(nc) as tc:
        with tc.tile_pool(name="persist", bufs=1) as pp:
            embT_aug = pp.tile([EMB + 1, rows], F32)
            ident24 = pp.tile([EMB, EMB], F32)
            negones = pp.tile([EMB, 1], F32)
            b4_t = pp.tile([EMB, 1], F32)
            a2all = pp.tile([128, tk], F32)
            make_identity(nc, ident24)
            nc.vector.memset(negones, -1.0)
            # row 24 of the augmented operands can't be written by compute
            # engines (partition start must be 32-aligned) -> DMA from a
            # host-supplied ones row.
            nc.sync.dma_start(out=embT_aug[EMB:EMB + 1, :], in_=onesr)
            nc.sync.dma_start(out=b4_t, in_=b4)

            # ---------------- Phase A: MLP ----------------
            with tc.tile_pool(name="wp", bufs=1) as wp, \
                 tc.tile_pool(name="xp", bufs=2) as xp, \
                 tc.tile_pool(name="hp", bufs=2) as hp, \
                 tc.tile_pool(name="gp", bufs=2) as gp, \
                 tc.tile_pool(name="psA", bufs=2, space="PSUM") as psA:

                def gelu_cast(hi_ap, lo_ap, in_ps, bias_ap):
                    # gelu -> fp32 transient, then split into fp16 hi + lo
                    ht = gp.tile([128, MT], F32, tag="hf")
                    gelu_op(ht, in_ps, bias_ap)
                    nc.vector.tensor_copy(hi_ap, ht)
                    ht32 = gp.tile([128, MT], F32, tag="hi32")
                    nc.vector.tensor_copy(ht32, hi_ap)
                    nc.vector.tensor_tensor(lo_ap, ht, ht32, ALU.subtract)

                def gelu_op(out_ap, in_ps, bias_ap):
                    if not gelu_decomp:
                        nc.scalar.activation(out_ap, in_ps, AF.Gelu_apprx_tanh,
                                             bias=bias_ap)
                        return
                    x = gp.tile([128, MT], F32, tag="gx")
                    nc.vector.tensor_tensor(
                        x, in_ps, bias_ap.to_broadcast([128, MT]), ALU.add)
                    t1 = gp.tile([128, MT], F32, tag="gt")
                    nc.vector.tensor_tensor(t1, x, x, ALU.mult)
                    nc.vector.tensor_scalar(t1, t1, GELU_K, GELU_C,
                                            ALU.mult, ALU.add)
                    nc.vector.tensor_tensor(t1, x, t1, ALU.mult)
                    th = gp.tile([128, MT], F32, tag="gth")
                    nc.scalar.activation(th, t1, AF.Tanh)
                    nc.vector.tensor_scalar(th, th, 0.5, 0.5, ALU.mult, ALU.add)
                    nc.vector.tensor_tensor(out_ap, x, th, ALU.mult)
                w1_t = wp.tile([128, 2, H], F32)
                w2hi_t = wp.tile([128, KC, H], F16)
                w2lo_t = wp.tile([128, KC, H], F16)
                w3hi_t = wp.tile([128, KC, H], F16)
                w3lo_t = wp.tile([128, KC, H], F16)
                w4hi_t = wp.tile([128, KC, EMB], F16)
                w4lo_t = wp.tile([128, KC, EMB], F16)
                b1_t = wp.tile([128, KC], F32)
                b2_t = wp.tile([128, KC], F32)
                b3_t = wp.tile([128, KC], F32)
                # w1+biases on the sync HWDGE queue (needed for tile 0);
                # bulky w2/w3/w4 via gpsimd SWDGE so the first x-tile DMA
                # isn't stuck behind 12MB on the same queue.
                nc.sync.dma_start(out=w1_t, in_=w1.rearrange("(c p) m -> p c m", p=128))
                nc.sync.dma_start(out=b1_t, in_=b1)
                nc.sync.dma_start(out=b2_t, in_=b2)
                nc.sync.dma_start(out=b3_t, in_=b3)
                for dst, src in ((w2hi_t, w2hi), (w2lo_t, w2lo),
                                 (w3hi_t, w3hi), (w3lo_t, w3lo),
                                 (w4hi_t, w4hi), (w4lo_t, w4lo)):
                    nc.gpsimd.dma_start(out=dst, in_=src.rearrange("(c p) m -> p c m", p=128))

                xT_r = xT.rearrange("(c p) n -> p c n", p=128)

                for t in range(nmt):
                    cols = slice(t * MT, (t + 1) * MT)
                    x_t = xp.tile([128, 2, MT], F32, tag="x")
                    nc.sync.dma_start(out=x_t, in_=xT_r[:, :, cols])
                    hAhi = hp.tile([128, KC, MT], F16, tag="hAhi")
                    hAlo = hp.tile([128, KC, MT], F16, tag="hAlo")
                    hBhi = hp.tile([128, KC, MT], F16, tag="hBhi")
                    hBlo = hp.tile([128, KC, MT], F16, tag="hBlo")

                    # L1: x(2 chunks) -> hA (gelu)
                    for g in range(2):
                        ps = psA.tile([128, 4, 512], F32, tag="mm")
                        for mh in range(4):
                            m = g * 4 + mh
                            for k in range(2):
                                nc.tensor.matmul(ps[:, mh, 0:MT],
                                                 w1_t[:, k, m * 128:(m + 1) * 128],
                                                 x_t[:, k, :],
                                                 start=(k == 0), stop=(k == 1))
                            gelu_cast(hAhi[:, m, :], hAlo[:, m, :], ps[:, mh, 0:MT], b1_t[:, m:m + 1])
                    # L2: hA -> hB
                    for g in range(2):
                        ps = psA.tile([128, 4, 512], F32, tag="mm")
                        for mh in range(4):
                            m = g * 4 + mh
                            msl = slice(m * 128, (m + 1) * 128)
                            parts = [(w2hi_t, hAhi), (w2lo_t, hAhi), (w2hi_t, hAlo)]
                            if t < 3:
                                parts.append((w2lo_t, hAlo))
                            for pi, (wt, ht_) in enumerate(parts):
                                for k in range(KC):
                                    nc.tensor.matmul(
                                        ps[:, mh, 0:MT], wt[:, k, msl], ht_[:, k, :],
                                        start=(pi == 0 and k == 0),
                                        stop=(pi == len(parts) - 1 and k == KC - 1))
                            gelu_cast(hBhi[:, m, :], hBlo[:, m, :], ps[:, mh, 0:MT], b2_t[:, m:m + 1])
                    # L3: hB -> hC (new hA-tagged tile)
                    hChi = hp.tile([128, KC, MT], F16, tag="hAhi")
                    hClo = hp.tile([128, KC, MT], F16, tag="hAlo")
                    for g in range(2):
                        ps = psA.tile([128, 4, 512], F32, tag="mm")
                        for mh in range(4):
                            m = g * 4 + mh
                            msl = slice(m * 128, (m + 1) * 128)
                            parts = [(w3hi_t, hBhi), (w3lo_t, hBhi), (w3hi_t, hBlo)]
                            if t < 3:
                                parts.append((w3lo_t, hBlo))
                            for pi, (wt, ht_) in enumerate(parts):
                                for k in range(KC):
                                    nc.tensor.matmul(
                                        ps[:, mh, 0:MT], wt[:, k, msl], ht_[:, k, :],
                                        start=(pi == 0 and k == 0),
                                        stop=(pi == len(parts) - 1 and k == KC - 1))
                            gelu_cast(hChi[:, m, :], hClo[:, m, :], ps[:, mh, 0:MT], b3_t[:, m:m + 1])
                    # L4: hC -> embT rows 0..23 (+ b4, exact DVE add)
                    ps4 = psA.tile([EMB, MT], F32, tag="mm")
                    parts4 = [(w4hi_t, hChi), (w4lo_t, hChi), (w4hi_t, hClo)]
                    if t < 3:
                        parts4.append((w4lo_t, hClo))
                    for pi, (wt, ht_) in enumerate(parts4):
                        for k in range(KC):
                            nc.tensor.matmul(ps4, wt[:, k, :], ht_[:, k, :],
                                             start=(pi == 0 and k == 0),
                                             stop=(pi == len(parts4) - 1 and k == KC - 1))
                    nc.vector.tensor_tensor(embT_aug[0:EMB, cols], ps4,
                                            b4_t.to_broadcast([EMB, MT]), ALU.add)

            # ---------------- Phase B1: semb ----------------
            sembT = pp.tile([EMB, C], F32)
            sembT_aug = pp.tile([EMB + 1, C], F32)
            rhs_aug = pp.tile([EMB + 1, C], F32)
            b2cols = pp.tile([128, KC], F32)
            e1024_rm = pp.tile([128, KC, EMB], F32)
            with tc.tile_pool(name="ktp", bufs=2) as ktp, \
                 tc.tile_pool(name="sb1", bufs=1) as sb1, \
                 tc.tile_pool(name="ps1", bufs=1, space="PSUM") as ps1:
                e1024_ps = ps1.tile([128, KC, EMB], F32)
                for j in range(KC):
                    nc.tensor.transpose(e1024_ps[:, j, :],
                                        embT_aug[0:EMB, j * 128:(j + 1) * 128],
                                        ident24)
                nc.vector.tensor_copy(e1024_rm, e1024_ps)
                nc.sync.dma_start(
                    out=o_emb[0:SH, :].rearrange("(j p) f -> p j f", p=128),
                    in_=e1024_rm)

                ssumT_ps = ps1.tile([EMB, 2, 512], F32)
                for v in range(KC):
                    kt_t = ktp.tile([128, C], F32, tag="kt")
                    nc.sync.dma_start(out=kt_t, in_=kt[v * 128:(v + 1) * 128, :])
                    for half in range(2):
                        nc.tensor.matmul(ssumT_ps[:, half, :],
                                         e1024_rm[:, v, :],
                                         kt_t[:, half * 512:(half + 1) * 512],
                                         start=(v == 0), stop=(v == KC - 1))
                cntr_t = sb1.tile([EMB, C], F32)
                nc.sync.dma_start(out=cntr_t, in_=cntr)
                for half in range(2):
                    hs = slice(half * 512, (half + 1) * 512)
                    nc.vector.tensor_tensor(sembT[:, hs], ssumT_ps[:, half, :],
                                            cntr_t[:, hs], ALU.mult)

                # sembT_aug (stationary for s-graph): rows = sembT, ones row
                nc.vector.tensor_copy(sembT_aug[0:EMB, :], sembT)
                nc.sync.dma_start(out=sembT_aug[EMB:EMB + 1, :],
                                  in_=onesr[:, 0:C])
                # rhs_aug (moving): rows = 2*sembT, row 24 = -||semb||^2
                nc.vector.tensor_scalar(rhs_aug[0:EMB, :], sembT, 2.0, None,
                                        ALU.mult)
                sq = sb1.tile([EMB, C], F32)
                nc.vector.tensor_tensor(sq, sembT, sembT, ALU.mult)
                b2neg_ps = ps1.tile([1, 2, 512], F32)
                for half in range(2):
                    nc.tensor.matmul(b2neg_ps[:, half, :], negones,
                                     sq[:, half * 512:(half + 1) * 512],
                                     start=True, stop=True)
                b2stage = sb1.tile([1, C], F32)
                nc.vector.tensor_copy(
                    b2stage.rearrange("p (h x) -> p h x", h=2), b2neg_ps)
                nc.sync.dma_start(out=rhs_aug[EMB:EMB + 1, :], in_=b2stage)

                # semb row-major: for output + b2cols
                srm_ps = ps1.tile([128, KC, EMB], F32)
                for j in range(KC):
                    nc.tensor.transpose(srm_ps[:, j, :],
                                        sembT[:, j * 128:(j + 1) * 128], ident24)
                srm_sb = sb1.tile([128, KC, EMB], F32)
                nc.vector.tensor_copy(srm_sb, srm_ps)
                nc.sync.dma_start(
                    out=o_semb.rearrange("(j p) f -> p j f", p=128), in_=srm_sb)
                sc1 = sb1.tile([128, KC, EMB], F32)
                for j in range(KC):
                    nc.scalar.activation(sc1[:, j, :], srm_sb[:, j, :], AF.Square,
                                         accum_out=b2cols[:, j:j + 1])
                # a2 for shared tiles
                sc2 = sb1.tile([128, KC, EMB], F32)
                for j in range(KC):
                    nc.scalar.activation(sc2[:, j, :], e1024_rm[:, j, :], AF.Square,
                                         accum_out=a2all[:, j:j + 1])

            # ---------------- Phase B2: distance + topk ----------------
            with tc.tile_pool(name="mp", bufs=2) as mp, \
                 tc.tile_pool(name="ep", bufs=3) as ep, \
                 tc.tile_pool(name="tks", bufs=3) as tks, \
                 tc.tile_pool(name="ps2", bufs=2, space="PSUM") as ps2:
                for j in range(tk):
                    rowsl = slice(j * 128, (j + 1) * 128)
                    if j >= KC:
                        erm_ps = ps2.tile([128, EMB], F32, tag="erm")
                        nc.tensor.transpose(erm_ps,
                                            embT_aug[0:EMB, rowsl], ident24)
                        erm_sb = ep.tile([128, EMB], F32, tag="erm_sb")
                        nc.vector.tensor_copy(erm_sb, erm_ps)
                        nc.sync.dma_start(out=o_emb[rowsl, :], in_=erm_sb)
                        scr = ep.tile([128, EMB], F32, tag="sq")
                        nc.scalar.activation(scr, erm_sb, AF.Square,
                                             accum_out=a2all[:, j:j + 1])

                    m_ps = ps2.tile([128, 2, 512], F32, tag="m")
                    for half in range(2):
                        nc.tensor.matmul(m_ps[:, half, :],
                                         embT_aug[:, rowsl],
                                         rhs_aug[:, half * 512:(half + 1) * 512],
                                         start=True, stop=True)
                    m_sb = mp.tile([128, C], F32, tag="m_sb")
                    nc.vector.tensor_copy(
                        m_sb.rearrange("p (h x) -> p h x", h=2), m_ps)

                    if j < KC:
                        d2row = mp.tile([128, C], F32, tag="d2row")
                        nc.vector.tensor_tensor(
                            d2row, a2all[:, j:j + 1].to_broadcast([128, C]),
                            m_sb, ALU.subtract)
                        nc.sync.dma_start(out=o_d2c[rowsl, :], in_=d2row)

                    mv = tks.tile([128, 8], F32, tag="mv")
                    mi = tks.tile([128, 8], U32, tag="mi")
                    nc.vector.max(out=mv, in_=m_sb)
                    nc.vector.max_index(out=mi, in_max=mv, in_values=m_sb)
                    nc.sync.dma_start(out=o_midx[rowsl, :], in_=mi)
                    d2t = tks.tile([128, 8], F32, tag="d2t")
                    nc.vector.tensor_tensor(
                        d2t, a2all[:, j:j + 1].to_broadcast([128, 8]), mv,
                        ALU.subtract)
                    nc.sync.dma_start(out=o_d2[rowsl, :], in_=d2t)

                # ---------------- Phase B3: s-graph (redundant, all 8 chunks) ----
                for j in range(KC):
                    rowsl = slice(j * 128, (j + 1) * 128)
                    m_ps = ps2.tile([128, 2, 512], F32, tag="m")
                    for half in range(2):
                        nc.tensor.matmul(m_ps[:, half, :],
                                         sembT_aug[:, rowsl],
                                         rhs_aug[:, half * 512:(half + 1) * 512],
                                         start=True, stop=True)
                    m_sb = mp.tile([128, C], F32, tag="m_sb")
                    nc.vector.tensor_copy(
                        m_sb.rearrange("p (h x) -> p h x", h=2), m_ps)
                    mv1 = tks.tile([128, 8], F32, tag="mv")
                    mi1 = tks.tile([128, 8], U32, tag="mi")
                    nc.vector.max(out=mv1, in_=m_sb)
                    nc.vector.max_index(out=mi1, in_max=mv1, in_values=m_sb)
                    nc.sync.dma_start(out=o_sidx[rowsl, 0:8], in_=mi1)
                    scr2 = mp.tile([128, C], F32, tag="scr")
                    nc.vector.match_replace(out=scr2, in_to_replace=mv1,
                                            in_values=m_sb, imm_value=-1e30)
                    mv2 = tks.tile([128, 8], F32, tag="mv")
                    mi2 = tks.tile([128, 8], U32, tag="mi")
                    nc.vector.max(out=mv2, in_=scr2)
                    nc.vector.max_index(out=mi2, in_max=mv2, in_values=scr2)
                    nc.sync.dma_start(out=o_sidx[rowsl, 8:16], in_=mi2)
                    for half, mvh in ((0, mv1), (1, mv2)):
                        d2h = tks.tile([128, 8], F32, tag="d2t")
                        nc.vector.tensor_tensor(
                            d2h, b2cols[:, j:j + 1].to_broadcast([128, 8]), mvh,
                            ALU.subtract)
                        nc.sync.dma_start(
                            out=o_sd2[rowsl, half * 8:(half + 1) * 8], in_=d2h)

    nc.compile()
    return nc


def _host_prep(inputs, rows=R):
    nodes = np.ascontiguousarray(inputs["nodes"], dtype=np.float32)
    cluster = np.asarray(inputs["cluster"])
    c0 = cluster[0].astype(np.int64)
    c1 = cluster[1].astype(np.int64)

    own = rows - SH
    total_pad = SH + NCORES * own
    nodes_pad = np.zeros((total_pad, D), np.float32)
    nodes_pad[:N] = nodes
    nodesT = nodes_pad.T  # [D, total_pad] view

    kt_h = np.zeros((C, C), np.float32)
    np.add.at(kt_h, (c0, c1), 1.0)
    cnt = np.bincount(c1, minlength=C).astype(np.float32)
    cntr_h = np.ascontiguousarray(np.broadcast_to(
        (np.float32(1.0) / np.maximum(cnt, 1.0))[None, :], (EMB, C)))

    def chunked(b):
        return np.ascontiguousarray(
            np.asarray(b, np.float32).reshape(H // 128, 128).T)

    def split16(w):
        w = np.asarray(w, np.float32)
        hi = w.astype(np.float16)
        lo = (w - hi.astype(np.float32)).astype(np.float16)
        return np.ascontiguousarray(hi), np.ascontiguousarray(lo)

    w2hi_h, w2lo_h = split16(inputs["W2"])
    w3hi_h, w3lo_h = split16(inputs["W3"])
    w4hi_h, w4lo_h = split16(inputs["W4"])
    common = {
        "w1": np.ascontiguousarray(inputs["W1"], np.float32),
        "w2hi": w2hi_h, "w2lo": w2lo_h,
        "w3hi": w3hi_h, "w3lo": w3lo_h,
        "w4hi": w4hi_h, "w4lo": w4lo_h,
        "b1": chunked(inputs["b1"]),
        "b2": chunked(inputs["b2"]),
        "b3": chunked(inputs["b3"]),
        "b4": np.ascontiguousarray(
            np.asarray(inputs["b4"], np.float32).reshape(EMB, 1)),
        "kt": kt_h,
        "cntr": cntr_h,
        "onesr": np.ones((1, rows), np.float32),
    }
    in_maps = []
    for c in range(NCORES):
        shard = np.empty((D, rows), np.float32)
        shard[:, :SH] = nodesT[:, :SH]
        shard[:, SH:] = nodesT[:, SH + c * own: SH + (c + 1) * own]
        m = dict(common)
        m["xT"] = shard
        in_maps.append(m)
    return in_maps, c0, c1


def _host_post(results, c0, c1, rows=R):
    own = rows - SH
    emb = np.empty((N, EMB), np.float32)
    midx = np.empty((N, 5), np.int64)
    d2_5 = np.empty((N, 5), np.float32)
    r0 = results[0]
    emb[:SH] = r0["o_emb"][:SH]
    midx[:SH] = r0["o_midx"][:SH, :5].astype(np.int64)
    d2_5[:SH] = r0["o_d2"][:SH, :5]
    for c in range(NCORES):
        g0 = SH + c * own
        take = min(own, N - g0)
        if take <= 0:
            continue
        rc = results[c]
        emb[g0:g0 + take] = rc["o_emb"][SH:SH + take]
        midx[g0:g0 + take] = rc["o_midx"][SH:SH + take, :5].astype(np.int64)
        d2_5[g0:g0 + take] = rc["o_d2"][SH:SH + take, :5]

    semb = r0["o_semb"]
    D2c = r0["o_d2c"]
    sidx = r0["o_sidx"][:, :10].astype(np.int64)
    sd2 = r0["o_sd2"][:, :10]

    # --- b-graph (node -> cluster kNN + original cluster edges) ---
    logits_knn = (-np.log(np.maximum(d2_5, np.float32(EPS)))).astype(np.float32)
    w_knn = np.exp(logits_knn).astype(np.float32)
    denom = w_knn.sum(1, dtype=np.float32)
    d2_c = D2c[c0, c1]
    logits_c = (-np.log(np.maximum(d2_c, np.float32(EPS)))).astype(np.float32)
    w_c = np.exp(logits_c).astype(np.float32)
    np.add.at(denom, c0, w_c)
    nodes_rep = np.repeat(np.arange(N, dtype=np.int32), 5)
    bgraph = np.stack([
        np.concatenate([nodes_rep, c0.astype(np.int32)]),
        np.concatenate([midx.ravel().astype(np.int32), c1.astype(np.int32)])])
    bweights = np.concatenate([
        w_knn.ravel() / (denom[np.repeat(np.arange(N), 5)] + np.float32(EPS)),
        w_c / (denom[c0] + np.float32(EPS))]).astype(np.float32)[:, None]
    logits = np.concatenate([logits_knn.ravel(), logits_c]).astype(np.float32)

    # --- s-graph (cluster <-> cluster symmetric kNN) ---
    slog = (-np.log(np.maximum(sd2, np.float32(EPS)))).astype(np.float32)
    w_s = (1.0 / (1.0 + np.exp(-slog))).astype(np.float32)
    s_flat = np.repeat(np.arange(C, dtype=np.int32), 10)
    si_flat = sidx.ravel().astype(np.int32)
    denom_s = w_s.sum(1, dtype=np.float32)
    np.add.at(denom_s, si_flat, w_s.ravel().astype(np.float32))
    sgraph = np.stack([np.concatenate([s_flat, si_flat]),
                       np.concatenate([si_flat, s_flat])])
    sweights = np.concatenate([
        w_s.ravel() / (denom_s[s_flat] + np.float32(EPS)),
        w_s.ravel() / (denom_s[si_flat] + np.float32(EPS))
    ]).astype(np.float32)[:, None]

    mask = np.ones(bgraph.shape[1], dtype=bool)
    return (emb, semb, bgraph, bweights, sgraph, sweights, logits, mask)


_NC_CACHE = {}


def _get_nc(rows=R):
    if rows not in _NC_CACHE:
        _NC_CACHE[rows] = build_nc(rows)
    return _NC_CACHE[rows]


def kernel(**inputs):
    in_maps, c0, c1 = _host_prep(inputs)
    nc = _get_nc()
    res = run_bass_kernel_spmd(nc, in_maps, core_ids=list(range(NCORES)))
    return _host_post(res.results, c0, c1)


# revision 16
# speedup vs baseline: 1.0795x; 1.0795x over previous
"""Trainium2 Bass kernel for nn_Pooling_23974507446587 (gnn_message_passing).

Data-parallel over nodes on 8 NeuronCores. Each core processes the shared
first-1024 rows (needed for scatter_mean -> semb, replicated so no collective
is needed) plus its own 6144-row slice: 7168 rows/core. The MLP (99% of
FLOPs), the N x C distance matrix, the top-k selection, and semb are computed
on device in fp32; the host does index plumbing and the O(E) log/exp edge
normalization on device-produced squared distances.
"""
import numpy as np

import concourse.bacc as bacc
import concourse.bass as bass
import concourse.mybir as mybir
from concourse.tile import TileContext
from concourse.bass_utils import run_bass_kernel_spmd
from concourse.masks import make_identity

F32 = mybir.dt.float32
F16 = mybir.dt.float16
U32 = mybir.dt.uint32
AF = mybir.ActivationFunctionType
ALU = mybir.AluOpType

N = 50000
D = 256
H = 1024
EMB = 24
C = 1024
EPS = 1e-12
NCORES = 8
SH = 1024            # shared rows (cluster[0] < 1024)
OWN = 6144           # own rows per core
R = SH + OWN         # 7168 rows per core
MT = 448             # MLP node-tile width (PSUM bank = 512 fp32)
NMT = R // MT        # 16
TK = R // 128        # 56 topk tiles


GELU_C = float(np.sqrt(2.0 / np.pi))
GELU_K = GELU_C * 0.044715


def build_nc(rows=R, gelu_decomp=False):
    nmt = rows // MT
    tk = rows // 128
    nc = bacc.Bacc("TRN2", target_bir_lowering=False, debug=False,
                   enable_asserts=False, num_devices=NCORES)

    xT = nc.dram_tensor("xT", [D, rows], F32, kind="ExternalInput").ap()
    w1 = nc.dram_tensor("w1", [D, H], F32, kind="ExternalInput").ap()
    w2hi = nc.dram_tensor("w2hi", [H, H], F16, kind="ExternalInput").ap()
    w2lo = nc.dram_tensor("w2lo", [H, H], F16, kind="ExternalInput").ap()
    w3hi = nc.dram_tensor("w3hi", [H, H], F16, kind="ExternalInput").ap()
    w3lo = nc.dram_tensor("w3lo", [H, H], F16, kind="ExternalInput").ap()
    w4hi = nc.dram_tensor("w4hi", [H, EMB], F16, kind="ExternalInput").ap()
    w4lo = nc.dram_tensor("w4lo", [H, EMB], F16, kind="ExternalInput").ap()
    b1 = nc.dram_tensor("b1", [128, H // 128], F32, kind="ExternalInput").ap()
    b2 = nc.dram_tensor("b2", [128, H // 128], F32, kind="ExternalInput").ap()
    b3 = nc.dram_tensor("b3", [128, H // 128], F32, kind="ExternalInput").ap()
    b4 = nc.dram_tensor("b4", [EMB, 1], F32, kind="ExternalInput").ap()
    kt = nc.dram_tensor("kt", [C, C], F32, kind="ExternalInput").ap()
    cntr = nc.dram_tensor("cntr", [EMB, C], F32, kind="ExternalInput").ap()
    onesr = nc.dram_tensor("onesr", [1, rows], F32, kind="ExternalInput").ap()

    o_emb = nc.dram_tensor("o_emb", [rows, EMB], F32, kind="ExternalOutput").ap()
    o_semb = nc.dram_tensor("o_semb", [C, EMB], F32, kind="ExternalOutput").ap()
    o_midx = nc.dram_tensor("o_midx", [rows, 8], U32, kind="ExternalOutput").ap()
    o_d2 = nc.dram_tensor("o_d2", [rows, 8], F32, kind="ExternalOutput").ap()
    o_d2c = nc.dram_tensor("o_d2c", [C, C], F32, kind="ExternalOutput").ap()
    o_sidx = nc.dram_tensor("o_sidx", [C, 16], U32, kind="ExternalOutput").ap()
    o_sd2 = nc.dram_tensor("o_sd2", [C, 16], F32, kind="ExternalOutput").ap()

    KC = H // 128   # 8 feature chunks

    with TileContext(nc) as tc:
        with tc.tile_pool(name="persist", bufs=1) as pp:
            embT_aug = pp.tile([EMB + 1, rows], F32)
            ident24 = pp.tile([EMB, EMB], F32)
            negones = pp.tile([EMB, 1], F32)
            b4_t = pp.tile([EMB, 1], F32)
            a2all = pp.tile([128, tk], F32)
            make_identity(nc, ident24)
            nc.vector.memset(negones, -1.0)
            # row 24 of the augmented operands can't be written by compute
            # engines (partition start must be 32-aligned) -> DMA from a
            # host-supplied ones row.
            nc.sync.dma_start(out=embT_aug[EMB:EMB + 1, :], in_=onesr)
            nc.sync.dma_start(out=b4_t, in_=b4)

            # ---------------- Phase A: MLP ----------------
            with tc.tile_pool(name="wp", bufs=1) as wp, \
                 tc.tile_pool(name="xp", bufs=2) as xp, \
                 tc.tile_pool(name="hp", bufs=2) as hp, \
                 tc.tile_pool(name="gp", bufs=2) as gp, \
                 tc.tile_pool(name="psA", bufs=2, space="PSUM") as psA:

                def gelu_cast(hi_ap, lo_ap, in_ps, bias_ap):
                    # gelu -> fp32 transient, then split into fp16 hi + lo
                    ht = gp.tile([128, MT], F32, tag="hf")
                    gelu_op(ht, in_ps, bias_ap)
                    nc.vector.tensor_copy(hi_ap, ht)
                    ht32 = gp.tile([128, MT], F32, tag="hi32")
                    nc.vector.tensor_copy(ht32, hi_ap)
                    nc.vector.tensor_tensor(lo_ap, ht, ht32, ALU.subtract)

                def gelu_op(out_ap, in_ps, bias_ap):
                    if not gelu_decomp:
                        nc.scalar.activation(out_ap, in_ps, AF.Gelu_apprx_tanh,
                                             bias=bias_ap)
                        return
                    x = gp.tile([128, MT], F32, tag="gx")
                    nc.vector.tensor_tensor(
                        x, in_ps, bias_ap.to_broadcast([128, MT]), ALU.add)
                    t1 = gp.tile([128, MT], F32, tag="gt")
                    nc.vector.tensor_tensor(t1, x, x, ALU.mult)
                    nc.vector.tensor_scalar(t1, t1, GELU_K, GELU_C,
                                            ALU.mult, ALU.add)
                    nc.vector.tensor_tensor(t1, x, t1, ALU.mult)
                    th = gp.tile([128, MT], F32, tag="gth")
                    nc.scalar.activation(th, t1, AF.Tanh)
                    nc.vector.tensor_scalar(th, th, 0.5, 0.5, ALU.mult, ALU.add)
                    nc.vector.tensor_tensor(out_ap, x, th, ALU.mult)
                w1_t = wp.tile([128, 2, H], F32)
                w2hi_t = wp.tile([128, KC, H], F16)
                w2lo_t = wp.tile([128, KC, H], F16)
                w3hi_t = wp.tile([128, KC, H], F16)
                w3lo_t = wp.tile([128, KC, H], F16)
                w4hi_t = wp.tile([128, KC, EMB], F16)
                w4lo_t = wp.tile([128, KC, EMB], F16)
                b1_t = wp.tile([128, KC], F32)
                b2_t = wp.tile([128, KC], F32)
                b3_t = wp.tile([128, KC], F32)
                # w1+biases on the sync HWDGE queue (needed for tile 0);
                # bulky w2/w3/w4 via gpsimd SWDGE so the first x-tile DMA
                # isn't stuck behind 12MB on the same queue.
                nc.sync.dma_start(out=w1_t, in_=w1.rearrange("(c p) m -> p c m", p=128))
                nc.sync.dma_start(out=b1_t, in_=b1)
                nc.sync.dma_start(out=b2_t, in_=b2)
                nc.sync.dma_start(out=b3_t, in_=b3)
                for dst, src in ((w2hi_t, w2hi), (w2lo_t, w2lo),
                                 (w3hi_t, w3hi), (w3lo_t, w3lo),
                                 (w4hi_t, w4hi), (w4lo_t, w4lo)):
                    nc.gpsimd.dma_start(out=dst, in_=src.rearrange("(c p) m -> p c m", p=128))

                xT_r = xT.rearrange("(c p) n -> p c n", p=128)

                for t in range(nmt):
                    cols = slice(t * MT, (t + 1) * MT)
                    x_t = xp.tile([128, 2, MT], F32, tag="x")
                    nc.sync.dma_start(out=x_t, in_=xT_r[:, :, cols])
                    hAhi = hp.tile([128, KC, MT], F16, tag="hAhi")
                    hAlo = hp.tile([128, KC, MT], F16, tag="hAlo")
                    hBhi = hp.tile([128, KC, MT], F16, tag="hBhi")
                    hBlo = hp.tile([128, KC, MT], F16, tag="hBlo")

                    # L1: x(2 chunks) -> hA (gelu)
                    for g in range(2):
                        ps = psA.tile([128, 4, 512], F32, tag="mm")
                        for mh in range(4):
                            m = g * 4 + mh
                            for k in range(2):
                                nc.tensor.matmul(ps[:, mh, 0:MT],
                                                 w1_t[:, k, m * 128:(m + 1) * 128],
                                                 x_t[:, k, :],
                                                 start=(k == 0), stop=(k == 1))
                            gelu_cast(hAhi[:, m, :], hAlo[:, m, :], ps[:, mh, 0:MT], b1_t[:, m:m + 1])
                    # L2: hA -> hB
                    for g in range(2):
                        ps = psA.tile([128, 4, 512], F32, tag="mm")
                        for mh in range(4):
                            m = g * 4 + mh
                            msl = slice(m * 128, (m + 1) * 128)
                            parts = [(w2hi_t, hAhi), (w2lo_t, hAhi), (w2hi_t, hAlo)]
                            for pi, (wt, ht_) in enumerate(parts):
                                for k in range(KC):
                                    nc.tensor.matmul(
                                        ps[:, mh, 0:MT], wt[:, k, msl], ht_[:, k, :],
                                        start=(pi == 0 and k == 0),
                                        stop=(pi == 2 and k == KC - 1))
                            gelu_cast(hBhi[:, m, :], hBlo[:, m, :], ps[:, mh, 0:MT], b2_t[:, m:m + 1])
                    # L3: hB -> hC (new hA-tagged tile)
                    hChi = hp.tile([128, KC, MT], F16, tag="hAhi")
                    hClo = hp.tile([128, KC, MT], F16, tag="hAlo")
                    for g in range(2):
                        ps = psA.tile([128, 4, 512], F32, tag="mm")
                        for mh in range(4):
                            m = g * 4 + mh
                            msl = slice(m * 128, (m + 1) * 128)
                            parts = [(w3hi_t, hBhi), (w3lo_t, hBhi), (w3hi_t, hBlo)]
                            for pi, (wt, ht_) in enumerate(parts):
                                for k in range(KC):
                                    nc.tensor.matmul(
                                        ps[:, mh, 0:MT], wt[:, k, msl], ht_[:, k, :],
                                        start=(pi == 0 and k == 0),
                                        stop=(pi == 2 and k == KC - 1))
                            gelu_cast(hChi[:, m, :], hClo[:, m, :], ps[:, mh, 0:MT], b3_t[:, m:m + 1])
                    # L4: hC -> embT rows 0..23 (+ b4, exact DVE add)
                    ps4 = psA.tile([EMB, MT], F32, tag="mm")
                    parts4 = [(w4hi_t, hChi), (w4lo_t, hChi), (w4hi_t, hClo)]
                    for pi, (wt, ht_) in enumerate(parts4):
                        for k in range(KC):
                            nc.tensor.matmul(ps4, wt[:, k, :], ht_[:, k, :],
                                             start=(pi == 0 and k == 0),
                                             stop=(pi == 2 and k == KC - 1))
                    nc.vector.tensor_tensor(embT_aug[0:EMB, cols], ps4,
                                            b4_t.to_broadcast([EMB, MT]), ALU.add)

            # ---------------- Phase B1: semb ----------------
            sembT = pp.tile([EMB, C], F32)
            sembT_aug = pp.tile([EMB + 1, C], F32)
            rhs_aug = pp.tile([EMB + 1, C], F32)
            b2cols = pp.tile([128, KC], F32)
            e1024_rm = pp.tile([128, KC, EMB], F32)
            with tc.tile_pool(name="ktp", bufs=2) as ktp, \
                 tc.tile_pool(name="sb1", bufs=1) as sb1, \
                 tc.tile_pool(name="ps1", bufs=1, space="PSUM") as ps1:
                e1024_ps = ps1.tile([128, KC, EMB], F32)
                for j in range(KC):
                    nc.tensor.transpose(e1024_ps[:, j, :],
                                        embT_aug[0:EMB, j * 128:(j + 1) * 128],
                                        ident24)
                nc.vector.tensor_copy(e1024_rm, e1024_ps)
                nc.sync.dma_start(
                    out=o_emb[0:SH, :].rearrange("(j p) f -> p j f", p=128),
                    in_=e1024_rm)

                ssumT_ps = ps1.tile([EMB, 2, 512], F32)
                for v in range(KC):
                    kt_t = ktp.tile([128, C], F32, tag="kt")
                    nc.sync.dma_start(out=kt_t, in_=kt[v * 128:(v + 1) * 128, :])
                    for half in range(2):
                        nc.tensor.matmul(ssumT_ps[:, half, :],
                                         e1024_rm[:, v, :],
                                         kt_t[:, half * 512:(half + 1) * 512],
                                         start=(v == 0), stop=(v == KC - 1))
                cntr_t = sb1.tile([EMB, C], F32)
                nc.sync.dma_start(out=cntr_t, in_=cntr)
                for half in range(2):
                    hs = slice(half * 512, (half + 1) * 512)
                    nc.vector.tensor_tensor(sembT[:, hs], ssumT_ps[:, half, :],
                                            cntr_t[:, hs], ALU.mult)

                # sembT_aug (stationary for s-graph): rows = sembT, ones row
                nc.vector.tensor_copy(sembT_aug[0:EMB, :], sembT)
                nc.sync.dma_start(out=sembT_aug[EMB:EMB + 1, :],
                                  in_=onesr[:, 0:C])
                # rhs_aug (moving): rows = 2*sembT, row 24 = -||semb||^2
                nc.vector.tensor_scalar(rhs_aug[0:EMB, :], sembT, 2.0, None,
                                        ALU.mult)
                sq = sb1.tile([EMB, C], F32)
                nc.vector.tensor_tensor(sq, sembT, sembT, ALU.mult)
                b2neg_ps = ps1.tile([1, 2, 512], F32)
                for half in range(2):
                    nc.tensor.matmul(b2neg_ps[:, half, :], negones,
                                     sq[:, half * 512:(half + 1) * 512],
                                     start=True, stop=True)
                b2stage = sb1.tile([1, C], F32)
                nc.vector.tensor_copy(
                    b2stage.rearrange("p (h x) -> p h x", h=2), b2neg_ps)
                nc.sync.dma_start(out=rhs_aug[EMB:EMB + 1, :], in_=b2stage)

                # semb row-major: for output + b2cols
                srm_ps = ps1.tile([128, KC, EMB], F32)
                for j in range(KC):
                    nc.tensor.transpose(srm_ps[:, j, :],
                                        sembT[:, j * 128:(j + 1) * 128], ident24)
                srm_sb = sb1.tile([128, KC, EMB], F32)
                nc.vector.tensor_copy(srm_sb, srm_ps)
                nc.sync.dma_start(
                    out=o_semb.rearrange("(j p) f -> p j f", p=128), in_=srm_sb)
                sc1 = sb1.tile([128, KC, EMB], F32)
                for j in range(KC):
                    nc.scalar.activation(sc1[:, j, :], srm_sb[:, j, :], AF.Square,
                                         accum_out=b2cols[:, j:j + 1])
                # a2 for shared tiles
                sc2 = sb1.tile([128, KC, EMB], F32)
                for j in range(KC):
                    nc.scalar.activation(sc2[:, j, :], e1024_rm[:, j, :], AF.Square,
                                         accum_out=a2all[:, j:j + 1])

            # ---------------- Phase B2: distance + topk ----------------
            with tc.tile_pool(name="mp", bufs=2) as mp, \
                 tc.tile_pool(name="ep", bufs=3) as ep, \
                 tc.tile_pool(name="tks", bufs=3) as tks, \
                 tc.tile_pool(name="ps2", bufs=2, space="PSUM") as ps2:
                for j in range(tk):
                    rowsl = slice(j * 128, (j + 1) * 128)
                    if j >= KC:
                        erm_ps = ps2.tile([128, EMB], F32, tag="erm")
                        nc.tensor.transpose(erm_ps,
                                            embT_aug[0:EMB, rowsl], ident24)
                        erm_sb = ep.tile([128, EMB], F32, tag="erm_sb")
                        nc.vector.tensor_copy(erm_sb, erm_ps)
                        nc.sync.dma_start(out=o_emb[rowsl, :], in_=erm_sb)
                        scr = ep.tile([128, EMB], F32, tag="sq")
                        nc.scalar.activation(scr, erm_sb, AF.Square,
                                             accum_out=a2all[:, j:j + 1])

                    m_ps = ps2.tile([128, 2, 512], F32, tag="m")
                    for half in range(2):
                        nc.tensor.matmul(m_ps[:, half, :],
                                         embT_aug[:, rowsl],
                                         rhs_aug[:, half * 512:(half + 1) * 512],
                                         start=True, stop=True)
                    m_sb = mp.tile([128, C], F32, tag="m_sb")
                    nc.vector.tensor_copy(
                        m_sb.rearrange("p (h x) -> p h x", h=2), m_ps)

                    if j < KC:
                        d2row = mp.tile([128, C], F32, tag="d2row")
                        nc.vector.tensor_tensor(
                            d2row, a2all[:, j:j + 1].to_broadcast([128, C]),
                            m_sb, ALU.subtract)
                        nc.sync.dma_start(out=o_d2c[rowsl, :], in_=d2row)

                    mv = tks.tile([128, 8], F32, tag="mv")
                    mi = tks.tile([128, 8], U32, tag="mi")
                    nc.vector.max(out=mv, in_=m_sb)
                    nc.vector.max_index(out=mi, in_max=mv, in_values=m_sb)
                    nc.sync.dma_start(out=o_midx[rowsl, :], in_=mi)
                    d2t = tks.tile([128, 8], F32, tag="d2t")
                    nc.vector.tensor_tensor(
                        d2t, a2all[:, j:j + 1].to_broadcast([128, 8]), mv,
                        ALU.subtract)
                    nc.sync.dma_start(out=o_d2[rowsl, :], in_=d2t)

                # ---------------- Phase B3: s-graph (redundant, all 8 chunks) ----
                for j in range(KC):
                    rowsl = slice(j * 128, (j + 1) * 128)
                    m_ps = ps2.tile([128, 2, 512], F32, tag="m")
                    for half in range(2):
                        nc.tensor.matmul(m_ps[:, half, :],
                                         sembT_aug[:, rowsl],
                                         rhs_aug[:, half * 512:(half + 1) * 512],
                                         start=True, stop=True)
                    m_sb = mp.tile([128, C], F32, tag="m_sb")
                    nc.vector.tensor_copy(
                        m_sb.rearrange("p (h x) -> p h x", h=2), m_ps)
                    mv1 = tks.tile([128, 8], F32, tag="mv")
                    mi1 = tks.tile([128, 8], U32, tag="mi")
                    nc.vector.max(out=mv1, in_=m_sb)
                    nc.vector.max_index(out=mi1, in_max=mv1, in_values=m_sb)
                    nc.sync.dma_start(out=o_sidx[rowsl, 0:8], in_=mi1)
                    scr2 = mp.tile([128, C], F32, tag="scr")
                    nc.vector.match_replace(out=scr2, in_to_replace=mv1,
                                            in_values=m_sb, imm_value=-1e30)
                    mv2 = tks.tile([128, 8], F32, tag="mv")
                    mi2 = tks.tile([128, 8], U32, tag="mi")
                    nc.vector.max(out=mv2, in_=scr2)
                    nc.vector.max_index(out=mi2, in_max=mv2, in_values=scr2)
                    nc.sync.dma_start(out=o_sidx[rowsl, 8:16], in_=mi2)
                    for half, mvh in ((0, mv1), (1, mv2)):
                        d2h = tks.tile([128, 8], F32, tag="d2t")
                        nc.vector.tensor_tensor(
                            d2h, b2cols[:, j:j + 1].to_broadcast([128, 8]), mvh,
                            ALU.subtract)
                        nc.sync.dma_start(
                            out=o_sd2[rowsl, half * 8:(half + 1) * 8], in_=d2h)

    nc.compile()
    return nc


def _host_prep(inputs, rows=R):
    nodes = np.ascontiguousarray(inputs["nodes"], dtype=np.float32)
    cluster = np.asarray(inputs["cluster"])
    c0 = cluster[0].astype(np.int64)
    c1 = cluster[1].astype(np.int64)

    own = rows - SH
    total_pad = SH + NCORES * own
    nodes_pad = np.zeros((total_pad, D), np.float32)
    nodes_pad[:N] = nodes
    nodesT = nodes_pad.T  # [D, total_pad] view

    kt_h = np.zeros((C, C), np.float32)
    np.add.at(kt_h, (c0, c1), 1.0)
    cnt = np.bincount(c1, minlength=C).astype(np.float32)
    cntr_h = np.ascontiguousarray(np.broadcast_to(
        (np.float32(1.0) / np.maximum(cnt, 1.0))[None, :], (EMB, C)))

    def chunked(b):
        return np.ascontiguousarray(
            np.asarray(b, np.float32).reshape(H // 128, 128).T)

    def split16(w):
        w = np.asarray(w, np.float32)
        hi = w.astype(np.float16)
        lo = (w - hi.astype(np.float32)).astype(np.float16)
        return np.ascontiguousarray(hi), np.ascontiguousarray(lo)

    w2hi_h, w2lo_h = split16(inputs["W2"])
    w3hi_h, w3lo_h = split16(inputs["W3"])
    w4hi_h, w4lo_h = split16(inputs["W4"])
    common = {
        "w1": np.ascontiguousarray(inputs["W1"], np.float32),
        "w2hi": w2hi_h, "w2lo": w2lo_h,
        "w3hi": w3hi_h, "w3lo": w3lo_h,
        "w4hi": w4hi_h, "w4lo": w4lo_h,
        "b1": chunked(inputs["b1"]),
        "b2": chunked(inputs["b2"]),
        "b3": chunked(inputs["b3"]),
        "b4": np.ascontiguousarray(
            np.asarray(inputs["b4"], np.float32).reshape(EMB, 1)),
        "kt": kt_h,
        "cntr": cntr_h,
        "onesr": np.ones((1, rows), np.float32),
    }
    in_maps = []
    for c in range(NCORES):
        shard = np.empty((D, rows), np.float32)
        shard[:, :SH] = nodesT[:, :SH]
        shard[:, SH:] = nodesT[:, SH + c * own: SH + (c + 1) * own]
        m = dict(common)
        m["xT"] = shard
        in_maps.append(m)
    return in_maps, c0, c1


def _host_post(results, c0, c1, rows=R):
    own = rows - SH
    emb = np.empty((N, EMB), np.float32)
    midx = np.empty((N, 5), np.int64)
    d2_5 = np.empty((N, 5), np.float32)
    r0 = results[0]
    emb[:SH] = r0["o_emb"][:SH]
    midx[:SH] = r0["o_midx"][:SH, :5].astype(np.int64)
    d2_5[:SH] = r0["o_d2"][:SH, :5]
    for c in range(NCORES):
        g0 = SH + c * own
        take = min(own, N - g0)
        if take <= 0:
            continue
        rc = results[c]
        emb[g0:g0 + take] = rc["o_emb"][SH:SH + take]
        midx[g0:g0 + take] = rc["o_midx"][SH:SH + take, :5].astype(np.int64)
        d2_5[g0:g0 + take] = rc["o_d2"][SH:SH + take, :5]

    semb = r0["o_semb"]
    D2c = r0["o_d2c"]
    sidx = r0["o_sidx"][:, :10].astype(np.int64)
    sd2 = r0["o_sd2"][:, :10]

    # --- b-graph (node -> cluster kNN + original cluster edges) ---
    logits_knn = (-np.log(np.maximum(d2_5, np.float32(EPS)))).astype(np.float32)
    w_knn = np.exp(logits_knn).astype(np.float32)
    denom = w_knn.sum(1, dtype=np.float32)
    d2_c = D2c[c0, c1]
    logits_c = (-np.log(np.maximum(d2_c, np.float32(EPS)))).astype(np.float32)
    w_c = np.exp(logits_c).astype(np.float32)
    np.add.at(denom, c0, w_c)
    nodes_rep = np.repeat(np.arange(N, dtype=np.int32), 5)
    bgraph = np.stack([
        np.concatenate([nodes_rep, c0.astype(np.int32)]),
        np.concatenate([midx.ravel().astype(np.int32), c1.astype(np.int32)])])
    bweights = np.concatenate([
        w_knn.ravel() / (denom[np.repeat(np.arange(N), 5)] + np.float32(EPS)),
        w_c / (denom[c0] + np.float32(EPS))]).astype(np.float32)[:, None]
    logits = np.concatenate([logits_knn.ravel(), logits_c]).astype(np.float32)

    # --- s-graph (cluster <-> cluster symmetric kNN) ---
    slog = (-np.log(np.maximum(sd2, np.float32(EPS)))).astype(np.float32)
    w_s = (1.0 / (1.0 + np.exp(-slog))).astype(np.float32)
    s_flat = np.repeat(np.arange(C, dtype=np.int32), 10)
    si_flat = sidx.ravel().astype(np.int32)
    denom_s = w_s.sum(1, dtype=np.float32)
    np.add.at(denom_s, si_flat, w_s.ravel().astype(np.float32))
    sgraph = np.stack([np.concatenate([s_flat, si_flat]),
                       np.concatenate([si_flat, s_flat])])
    sweights = np.concatenate([
        w_s.ravel() / (denom_s[s_flat] + np.float32(EPS)),
        w_s.ravel() / (denom_s[si_flat] + np.float32(EPS))
    ]).astype(np.float32)[:, None]

    mask = np.ones(bgraph.shape[1], dtype=bool)
    return (emb, semb, bgraph, bweights, sgraph, sweights, logits, mask)


_NC_CACHE = {}


def _get_nc(rows=R):
    if rows not in _NC_CACHE:
        _NC_CACHE[rows] = build_nc(rows)
    return _NC_CACHE[rows]


def kernel(**inputs):
    in_maps, c0, c1 = _host_prep(inputs)
    nc = _get_nc()
    res = run_bass_kernel_spmd(nc, in_maps, core_ids=list(range(NCORES)))
    return _host_post(res.results, c0, c1)
